# revision 36
# baseline (speedup 1.0000x reference)
"""AfmoeTokenChoiceRouter kernel for 8x Trainium2 NeuronCores.

Data-parallel over tokens: each of the 8 cores handles 2048 tokens.

Precision scheme (3 bytes/element of x instead of 4):
  x  = xh (fp16) + r,   r shipped as xl8 = e4m3(r * 2^16)     [1 byte]
  w  = wh (fp16) + wl (fp16)  [replicated, tiny]
  w8 = e4m3(w * 2^11)   [replicated, tiny]
The fp16 stream is shipped pre-scaled (xh*2^13, w*2^14 -- exact exponent
shifts) so its products land at x*w*2^27, the SAME scale as the fp8 stream's
(r*2^16)*(w*2^11): both streams accumulate into ONE PSUM region and the
2^-27 folds into the combine constant. Top-8 selection matches a pure-fp32
reference on all but ~4 near-tie tokens in 16384 (L2 idx rel err ~5e-3),
while HBM traffic drops from 16.8 MB to 12.6 MB per core.

Per core pipeline (supertile blocks per SCHEDULE, host-packed so every DMA
is a contiguous full-rate burst; tapering tail keeps the serial drain short):
  - DMA: xh (fp16) on the sync HWDGE queue, xl8 (fp8) on the ACT queue
  - PE per block: 16 fp16 matmuls ([wh_c|wl_c] stationary x xh_c -> psum
    rows 0:64 wh terms + 2^-13-aligned, 64:128 wl terms), then 8 fp8
    DoubleRow matmuls (2 k-chunks each, w8 pairs stationary) accumulating
    the residual into rows 0:64 of the same psum. Grouping the fp8 stream
    after the fp16 stream (one dtype/stationary switch per block) measured
    ~1.6x faster than interleaving them per-chunk on hardware.
  - PE per 128-token tile: one "J-matmul" (data block stationary, constant
    jA = 2^-27*[I64;I64] moving) fuses the back-transpose, the hi+lo row
    fold and the 2^-27 descale in a single instruction -> l_ps [tok, 64]
  - the J-matmuls + topk of block i are emitted AFTER block i+1's GEMM
    matmuls (_DEFER_J): the PE engine queue is in-order, so without the
    deferral each block's J-matmuls stall the PE behind the ACT
    PSUM->SBUF copy round-trip instead of running block i+1's matmuls
  - ACT sigmoid; DVE top-8: max8/max_index on biased scores, per-tile
    threshold mask (TimelineSim: batching the mask across tiles adds a
    cross-tile barrier and is a net loss), second max8 pass on masked
    unbiased scores, 8x8 index-match reorder directly on u32 indices
    (saves 2 copies/block; TimelineSim −0.7us), normalize, scale 2.5
  - outputs stored via the ACT HWDGE queue (15 head tiles early, 1-tile
    tail at the end, scores/indices tails split across the ACT/SP rings
    so their ~2us HBM write-completion latencies overlap); SWDGE
    descriptor generation (~6us per strided store) and per-block store
    dribble both measured as tail serializers. TimelineSim shows the
    drain (last 128-tok block's DMA->PE->copy->J->sigmoid->topk->store
    chain) is ~6.6us and is the main residual overhead beyond the DMA
    window; schedule taper + early stores exist to minimize it.
Outputs per core: scores [128, 16, 8] f32 and indices [128, 16, 8] u32 in
partition-major token order (token = 128*tile + partition), unpermuted on
the host.
"""

import contextlib as _contextlib

import numpy as np
import ml_dtypes

import concourse.bass as bass
import concourse.mybir as mybir
import concourse.tile as tile
import concourse.bass_utils as bass_utils
from concourse import bacc
from concourse.masks import make_identity

f32 = mybir.dt.float32
f16 = mybir.dt.float16
f8 = mybir.dt.float8e4
u32 = mybir.dt.uint32
Alu = mybir.AluOpType
Act = mybir.ActivationFunctionType
_nullctx = _contextlib.nullcontext

N_CORES = 8
T_FULL, H, E, TOPK = 16384, 2048, 64, 8
T_CORE = T_FULL // N_CORES          # 2048
N_TILES = T_CORE // 128             # 16
N_CH = H // 128                     # 16 contraction chunks
ROUTE_SCALE = 2.5
# supertile schedule; baked into the host packing (each block is stored
# contiguously so every DMA is a full-rate linear burst). Tapering tail keeps
# the post-last-DMA serial chain short.
SCHEDULE = [384, 384, 384, 384, 256, 128, 128]
XA = 16                             # xl8 = e4m3(r * 2^XA), max |val| = 128
WB = 11                             # w8 = e4m3(w * 2^WB), max |val| = 224
# the fp16 stream is shipped pre-scaled (xh*2^13, w*2^14) so its PSUM terms
# land at x*w*2^27 == the fp8 stream's r*2^16 * w*2^11 scale: both streams
# accumulate into ONE psum, and the 2^-27 folds into the combine constant.
XS, WS = 13, 14
CSCALE = 2.0 ** (-(XA + WB))
assert XS + WS == XA + WB


def router_body(tc, outs, ins, reps=1, skip_dma=False, skip_compute=False,
                n_terms=3, skip_topk=False):
    nc = tc.nc
    out_s_d, out_i_d = outs
    xh_d, xl_d, w2_d, w8_d, bias_d, jab_d = ins

    with (
        tc.tile_pool(name="const", bufs=1) as constp,
        tc.tile_pool(name="xin", bufs=globals().get('_XBUFS', 3)) as xpool,
        tc.tile_pool(name="persist", bufs=1) as pers,
        tc.tile_pool(name="scratch", bufs=globals().get('_SCRBUFS', 4)) as scr,
        tc.tile_pool(name="ps_lt", bufs=globals().get('_LTBUFS', 3), space="PSUM") as ps_lt,
        tc.tile_pool(name="ps_l", bufs=globals().get('_PLBUFS', 3), space="PSUM") as ps_l,
    ):
        # setup DMAs ride the ACT HWDGE queue so they don't delay the first
        # x pieces on the sync queue (HWDGE is FIFO per issuing engine).
        # w2 is split into pieces so the first chunks' matmuls can start
        # before the whole 512 KB stationary lands; bias/jab (needed only
        # ~8us in) are deferred until after the first xl block (see below).
        w2_sb = constp.tile([128, N_CH, 128], f16)
        w8_sb = constp.tile([128, N_CH, E], f8)
        w2_r = w2_d.rearrange("p (c e) -> p c e", e=128)
        wdef = globals().get('_W2_DEFER', 0)
        late_w = []
        for d0 in range(0, N_CH, 4):
            if wdef and d0 >= wdef:
                late_w.append((w2_sb[:, d0:d0 + 4, :], w2_r[:, d0:d0 + 4, :]))
            else:
                nc.scalar.dma_start(w2_sb[:, d0:d0 + 4, :], w2_r[:, d0:d0 + 4, :])
        if wdef:
            late_w.append((w8_sb[:], w8_d))
        else:
            nc.scalar.dma_start(w8_sb[:], w8_d)
        bias_sb = constp.tile([128, 1, E], f32)
        # combine constants: jA = [I64; I64], jB = 2^-27 * I64. Used as the
        # moving operand of per-tile "transpose" matmuls that fuse the
        # back-transpose with the hi+lo add and the scaled fp8-term add.
        jab_sb = constp.tile([128, 1, E], f32)
        setup_rest = late_w + [(bias_sb, bias_d), (jab_sb, jab_d)]
        if skip_dma:
            while setup_rest:
                sb, dr = setup_rest.pop(0)
                nc.scalar.dma_start(sb[:], dr)

        # persistent per-core tensors
        s_all = pers.tile([128, N_TILES, E], f32)      # sigmoid scores
        b_all = pers.tile([128, N_TILES, E], f32)      # biased scores
        vb_all = pers.tile([128, N_TILES, 8], f32)     # top8 of biased
        vs_all = pers.tile([128, N_TILES, 8], f32)     # top8 of masked s
        ib_all = pers.tile([128, N_TILES, 8], u32)     # indices (biased order)
        is_all = pers.tile([128, N_TILES, 8], u32)     # indices (s order)
        ibf = pers.tile([128, N_TILES, 8], f32)
        isf = pers.tile([128, N_TILES, 8], f32)
        out_s_sb = pers.tile([128, N_TILES, 8], f32)

        DMA_CH = globals().get('_DMA_CH_OVERRIDE', 4)   # h-chunks per xh piece
        DMA_CHL = globals().get('_DMA_CHL_OVERRIDE', 16)  # h-chunks per xl piece

        def supertile(pos, tok_st):
            tiles_ss = tok_st // 128
            t0 = pos // 128
            s4 = slice(t0, t0 + tiles_ss)
            xh_sb = xpool.tile([128, N_CH, tok_st], f16, tag="xh")
            xl_sb = xpool.tile([128, N_CH, tok_st], f8, tag="xl")
            foff = N_CH * pos
            xh_st = xh_d[:, foff:foff + N_CH * tok_st].rearrange(
                "p (c t) -> p c t", t=tok_st)
            xl_st = xl_d[:, foff:foff + N_CH * tok_st].rearrange(
                "p (c t) -> p c t", t=tok_st)
            if not skip_dma:
                bal = globals().get('_BAL_QUEUES', 0)
                xl_eng = nc.scalar if globals().get('_XL_ON_ACT', 1) else nc.sync
                if bal == 3:
                    # 3-ring split: xh pieces round-robin sync/ACT/pool-SWDGE,
                    # xl pieces round-robin the same three
                    engs = [nc.sync, nc.scalar, nc.gpsimd]
                    d0 = 0
                    pi = 0
                    while d0 < N_CH:
                        dn = min(DMA_CH, N_CH - d0)
                        engs[pi % 3].dma_start(
                            xh_sb[:, d0:d0 + dn, :], xh_st[:, d0:d0 + dn, :])
                        d0 += dn
                        pi += 1
                    d0 = 0
                    while d0 < N_CH:
                        dn = min(DMA_CHL // 2, N_CH - d0)
                        engs[pi % 3].dma_start(
                            xl_sb[:, d0:d0 + dn, :], xl_st[:, d0:d0 + dn, :])
                        d0 += dn
                        pi += 1
                else:
                    d0 = 0
                    pi = 0
                    while d0 < N_CH:
                        dn = min(DMA_CH, N_CH - d0)
                        eng = nc.sync if (not bal or pi % 2 == 0) else nc.scalar
                        eng.dma_start(xh_sb[:, d0:d0 + dn, :], xh_st[:, d0:d0 + dn, :])
                        d0 += dn
                        pi += 1
                    d0 = 0
                    pi = 0
                    while d0 < N_CH:
                        dn = min(DMA_CHL, N_CH - d0)
                        eng = xl_eng if (not bal or pi % 2 == 0) else nc.sync
                        eng.dma_start(xl_sb[:, d0:d0 + dn, :], xl_st[:, d0:d0 + dn, :])
                        d0 += dn
                        pi += 1
                while setup_rest:
                    sb, dr = setup_rest.pop(0)
                    nc.scalar.dma_start(sb[:], dr)
            else:
                # timing-ablation mode: mark x tiles written so the tile
                # framework doesn't assert on read-without-write
                nc.gpsimd.memset(xh_sb[:, 0, :1], 0)
                nc.gpsimd.memset(xl_sb[:, 0, :1], 0)
            if skip_compute:
                return

            # GEMM: psumA <- [wh_c|wl_c] fp16 x xh_c (rows 0:64 wh, 64:128 wl)
            #       psumB <- w8_c fp8 x xl8_c (rows 0:64)
            # both streams accumulate into one psum: fp16 terms into rows
            # 0:128 ([wh|wl] stationary), fp8 residual terms into rows 0:64
            # (same scale 2^27 by construction)
            lt_ps = ps_lt.tile([128, tok_st], f32, tag="lt")
            use_dr = globals().get('_DR', 1) and \
                tok_st >= globals().get('_DR_MIN_FD', 0)
            b_sep = globals().get('_BSEP', 1)
            for c in range(N_CH):
                last_a = c == N_CH - 1 and n_terms < 3
                nc.tensor.matmul(lt_ps[:], w2_sb[:, c, :], xh_sb[:, c, :],
                                 start=(c == 0), stop=last_a)
                if n_terms >= 3 and not use_dr and not b_sep:
                    nc.tensor.matmul(lt_ps[0:64, :], w8_sb[:, c, :],
                                     xl_sb[:, c, :],
                                     start=False, stop=(c == N_CH - 1))
            if n_terms >= 3 and not use_dr and b_sep:
                for c in range(N_CH):
                    nc.tensor.matmul(lt_ps[0:64, :], w8_sb[:, c, :],
                                     xl_sb[:, c, :],
                                     start=False, stop=(c == N_CH - 1))
            if n_terms >= 3 and use_dr:
                # fp8 DoubleRow: 2 contraction chunks per matmul
                # (out = sum_j lhsT[:, j, :].T @ rhs[:, j, :])
                for cc in range(N_CH // 2):
                    nc.tensor.matmul(
                        lt_ps[0:64, :], w8_sb[:, 2 * cc:2 * cc + 2, :],
                        xl_sb[:, 2 * cc:2 * cc + 2, :],
                        start=False, stop=(cc == N_CH // 2 - 1),
                        perf_mode=mybir.MatmulPerfMode.DoubleRow)
            if n_terms < 3:
                dummy = scr.tile([128, 1], f8, tag="dummy")
                nc.vector.tensor_copy(dummy[:], xl_sb[:, 0, :1])

            if globals().get('_DEFER_COPY', 0):
                return lt_ps
            return do_copy(tok_st, lt_ps)

        def do_copy(tok_st, lt_ps):
            lt_sb = scr.tile([128, tok_st], f32, tag="ltsb")
            cp = globals().get('_COPY_ENG', 'scalar')
            if cp == 'pool':
                nc.gpsimd.tensor_copy(lt_sb[:], lt_ps[:])
            elif cp == 'vector':
                nc.vector.tensor_copy(lt_sb[:], lt_ps[:])
            else:
                nc.scalar.copy(lt_sb[:], lt_ps[:])
            return lt_sb

        def finish_block(pos, tok_st, lt_sb, last=None):
            tiles_ss = tok_st // 128
            t0 = pos // 128
            s4 = slice(t0, t0 + tiles_ss)
            # fused back-transpose + combine: per 128-token block,
            #   l_ps[t, e] = sum_r lt[r, t] * jA[r, e] = 2^-27 * (hi + lo rows)
            # (the data block is the stationary, jA the 64-col moving)
            l_ps = ps_l.tile([128, tiles_ss, E], f32, tag="lps")
            for q in range(tiles_ss):
                qs = slice(q * 128, (q + 1) * 128)
                nc.tensor.matmul(l_ps[:, q, :], lt_sb[:, qs], jab_sb[:, 0, :],
                                 start=True, stop=True)

            s_sl = s_all[:, s4, :]
            nc.scalar.activation(s_sl, l_ps[:, :, :], Act.Sigmoid)
            if skip_topk:
                nc.vector.tensor_copy(out_s_sb[:, s4, :], s_sl[:, :, :8])
                nc.vector.tensor_copy(ib_all[:, s4, :], s_sl[:, :, 8:16])
                return
            b_sl = b_all[:, s4, :]
            beng = nc.gpsimd if globals().get('_BIAS_ON_POOL', 0) else nc.vector
            beng.tensor_tensor(
                out=b_sl, in0=s_sl,
                in1=bias_sb[:].broadcast_to([128, tiles_ss, E]),
                op=Alu.add,
            )

            for q in range(tiles_ss):
                i = t0 + q
                nc.vector.max(out=vb_all[:, i, :], in_=b_all[:, i, :])
                nc.vector.max_index(out=ib_all[:, i, :], in_max=vb_all[:, i, :],
                                    in_values=b_all[:, i, :])
            if last is not None:
                # the indices tail store needs only max_index output; issuing
                # it here lets its ~2us HBM write receipt overlap the rest of
                # the score chain (mask/2nd pass/normalize) of the last block
                od_i = out_i_d.rearrange("p (i k) -> p i k", k=8)
                nc.sync.dma_start(od_i[:, last:, :], ib_all[:, last:, :])

            # selected-expert masking: sarr = (b >= thr8) * s
            sarr = scr.tile([128, tiles_ss, E], f32, tag="sarr")
            if globals().get('_BATCH_MASK', 0):
                ge = scr.tile([128, tiles_ss, E], f32, tag="ge")
                nc.vector.tensor_tensor(
                    out=ge[:], in0=b_all[:, s4, :],
                    in1=vb_all[:, s4, 7:8].broadcast_to([128, tiles_ss, E]),
                    op=Alu.is_ge)
                nc.vector.tensor_tensor(
                    out=sarr[:], in0=ge[:], in1=s_all[:, s4, :], op=Alu.mult)
            else:
                for q in range(tiles_ss):
                    i = t0 + q
                    nc.vector.scalar_tensor_tensor(
                        out=sarr[:, q, :], in0=b_all[:, i, :],
                        scalar=vb_all[:, i, 7:8], in1=s_all[:, i, :],
                        op0=Alu.is_ge, op1=Alu.mult)

            for q in range(tiles_ss):
                i = t0 + q
                nc.vector.max(out=vs_all[:, i, :], in_=sarr[:, q, :])
                nc.vector.max_index(out=is_all[:, i, :], in_max=vs_all[:, i, :],
                                    in_values=sarr[:, q, :])

            # reorder vs_all (s-descending) into biased-rank order by idx match
            eeng = nc.gpsimd if globals().get('_EQ_ON_POOL', 0) else nc.vector
            eq = scr.tile([128, tiles_ss, 8, 8], f32, tag="eq")
            if globals().get('_EQ_U32', 1):
                eeng.tensor_tensor(
                    out=eq[:],
                    in0=ib_all[:, s4, :].broadcast_to([128, tiles_ss, 8, 8]),
                    in1=is_all[:, s4, :][:, :, None, :].broadcast_to(
                        [128, tiles_ss, 8, 8]),
                    op=Alu.is_equal,
                )
            else:
                nc.vector.tensor_copy(ibf[:, s4, :], ib_all[:, s4, :])
                nc.vector.tensor_copy(isf[:, s4, :], is_all[:, s4, :])
                eeng.tensor_tensor(
                    out=eq[:],
                    in0=ibf[:, s4, :].broadcast_to([128, tiles_ss, 8, 8]),
                    in1=isf[:, s4, :][:, :, None, :].broadcast_to(
                        [128, tiles_ss, 8, 8]),
                    op=Alu.is_equal,
                )
            g_sc = scr.tile([128, tiles_ss, 8, 8], f32, tag="g")
            eeng.tensor_tensor(
                out=g_sc[:], in0=eq[:],
                in1=vs_all[:, s4, :][:, :, None, :].broadcast_to(
                    [128, tiles_ss, 8, 8]),
                op=Alu.mult,
            )
            tsr = scr.tile([128, tiles_ss, 8], f32, tag="tsr")
            nc.vector.reduce_sum(out=tsr[:], in_=g_sc[:], axis=mybir.AxisListType.X)

            den = scr.tile([128, tiles_ss], f32, tag="den")
            nc.vector.reduce_sum(out=den[:], in_=vs_all[:, s4, :],
                                 axis=mybir.AxisListType.X)
            rec = scr.tile([128, tiles_ss], f32, tag="rec")
            nc.vector.reciprocal(rec[:], den[:])
            nc.vector.scalar_tensor_tensor(
                out=out_s_sb[:, s4, :], in0=tsr[:], scalar=ROUTE_SCALE,
                in1=rec[:].broadcast_to([128, tiles_ss, 8]),
                op0=Alu.mult, op1=Alu.mult,
            )
            if globals().get('_OUT_PER_ST', 0):
                od_s = out_s_d.rearrange("p (i k) -> p i k", k=8)
                od_i = out_i_d.rearrange("p (i k) -> p i k", k=8)
                nc.scalar.dma_start(od_s[:, s4, :], out_s_sb[:, s4, :])
                nc.scalar.dma_start(od_i[:, s4, :], ib_all[:, s4, :])

        schedule = globals().get('_SCHED', SCHEDULE)
        assert sum(schedule) == T_CORE

        def whole_pass():
            pos = 0
            pos_fin = 0
            tail0 = 0
            n_early = globals().get('_EARLY_TILES', 15)
            hi_last = globals().get('_HI_LAST', 0)
            defer = globals().get('_DEFER_J', 1)
            early_done = False
            pending = None          # (pos, tok_st, lt_sb) not yet finished
            store_out = not skip_compute and not skip_topk and \
                not globals().get('_OUT_PER_ST', 0)

            def maybe_early_store():
                nonlocal early_done, tail0
                if store_out and not early_done and pos_fin >= 128 * n_early:
                    # store the finished head tiles while the x stream still
                    # runs; only the short tail rides the final store pair
                    e = pos_fin // 128
                    od_s = out_s_d.rearrange("p (i k) -> p i k", k=8)
                    od_i = out_i_d.rearrange("p (i k) -> p i k", k=8)
                    nc.scalar.dma_start(od_s[:, 0:e, :], out_s_sb[:, 0:e, :])
                    nc.scalar.dma_start(od_i[:, 0:e, :], ib_all[:, 0:e, :])
                    early_done = True
                    tail0 = e

            warm = globals().get('_PE_WARM', 0)
            if warm and not skip_compute:
                # HAM keep-warm filler: PE idles ~5us at each iteration start
                # (all-engine loop barrier + first block's DMA), long enough
                # for the clock gate to re-throttle to 1.2 GHz. Issue dummy
                # matmuls with no DMA dependency to span the gap and hold the
                # 2.4 GHz clock. Sized to finish before the first block lands.
                wm_ps = ps_lt.tile([128, 128], f32, tag="lt")
                for _ in range(warm):
                    nc.tensor.matmul(wm_ps[:], w2_sb[:, 0, :], w2_sb[:, 0, :],
                                     start=True, stop=True)

            for bi, tok_st in enumerate(schedule):
                hp = hi_last and bi >= len(schedule) - hi_last
                with tc.high_priority() if hp else _nullctx():
                    lt_sb = supertile(pos, tok_st)
                if not skip_compute:
                    if defer:
                        if pending is not None:
                            p_pos, p_tok, p_lt = pending
                            if globals().get('_DEFER_COPY', 0):
                                p_lt = do_copy(p_tok, p_lt)
                            finish_block(p_pos, p_tok, p_lt)
                            pos_fin = p_pos + p_tok
                        pending = (pos, tok_st, lt_sb)
                    else:
                        finish_block(pos, tok_st, lt_sb)
                        pos_fin = pos + tok_st
                pos += tok_st
                maybe_early_store()
            idx_tail_done = None
            if pending is not None:
                p_pos, p_tok, p_lt = pending
                # if the stored tail lies entirely within the last block,
                # fire its indices store early (right after max_index)
                if store_out and globals().get('_EARLY_IDX_TAIL', 1) \
                        and n_early >= p_pos // 128:
                    idx_tail_done = max(p_pos // 128, n_early)
                with tc.high_priority() if globals().get('_HI_FLUSH', 0) \
                        else _nullctx():
                    if globals().get('_DEFER_COPY', 0):
                        p_lt = do_copy(p_tok, p_lt)
                    finish_block(p_pos, p_tok, p_lt, last=idx_tail_done)
                pos_fin = p_pos + p_tok
                maybe_early_store()
            if not skip_compute and not skip_topk and not globals().get('_OUT_PER_ST', 0):
                t0 = tail0 if early_done else 0
                od_s = out_s_d.rearrange("p (i k) -> p i k", k=8)
                od_i = out_i_d.rearrange("p (i k) -> p i k", k=8)
                ieng = nc.sync if globals().get('_STORE_SPLIT', 1) else nc.scalar
                nc.scalar.dma_start(od_s[:, t0:, :], out_s_sb[:, t0:, :])
                # indices tail may already be (partially) stored by the last
                # finish_block's early-idx store, which covered [ie, N_TILES)
                ie = idx_tail_done if idx_tail_done is not None else N_TILES
                if ie > t0:
                    ieng.dma_start(od_i[:, t0:ie, :], ib_all[:, t0:ie, :])

        if reps == 1:
            whole_pass()
        else:
            with tc.For_i(0, reps, 1):
                whole_pass()


def build_nc(reps=1, skip_dma=False, skip_compute=False, n_terms=3, skip_topk=False):
    nc = bacc.Bacc("TRN2", target_bir_lowering=False, debug=False)
    xh_d = nc.dram_tensor("xh_d", [128, N_CH * T_CORE], f16, kind="ExternalInput")
    xl_d = nc.dram_tensor("xl_d", [128, N_CH * T_CORE], f8, kind="ExternalInput")
    w2_d = nc.dram_tensor("w2_d", [128, N_CH * 128], f16, kind="ExternalInput")
    w8_d = nc.dram_tensor("w8_d", [128, N_CH * E], f8, kind="ExternalInput")
    bias_d = nc.dram_tensor("bias_d", [128, E], f32, kind="ExternalInput")
    jab_d = nc.dram_tensor("jab_d", [128, E], f32, kind="ExternalInput")
    out_s_d = nc.dram_tensor("out_s_d", [128, N_TILES * 8], f32, kind="ExternalOutput")
    out_i_d = nc.dram_tensor("out_i_d", [128, N_TILES * 8], u32, kind="ExternalOutput")

    with tile.TileContext(nc) as tc:
        router_body(
            tc,
            (out_s_d.ap(), out_i_d.ap()),
            (xh_d.ap(), xl_d.ap(), w2_d.ap(), w8_d.ap(), bias_d.ap(), jab_d.ap()),
            reps=reps, skip_dma=skip_dma, skip_compute=skip_compute,
            n_terms=n_terms, skip_topk=skip_topk,
        )
    nc.compile()
    return nc


def pack_x_shard(xT, dtype):
    """[H, T_CORE] -> [128, N_CH*T_CORE] with each SCHEDULE block stored
    contiguously: out[p, N_CH*pos + c*tok_st + t] = xT[c*128 + p, pos + t]."""
    v = xT.reshape(N_CH, 128, T_CORE)
    blocks = []
    pos = 0
    for tok_st in globals().get('_SCHED', SCHEDULE):
        blk = v[:, :, pos:pos + tok_st]            # [N_CH, 128, tok_st]
        blocks.append(blk.transpose(1, 0, 2).reshape(128, N_CH * tok_st))
        pos += tok_st
    return np.ascontiguousarray(np.concatenate(blocks, axis=1)).astype(dtype)


def pack_w2(wh, wl):
    """wh/wl [E, H] fp16 -> [128, N_CH*128] with wh in cols 0:64, wl in 64:128
    of each chunk: out[p, c*128 + e] = (wh if e < E else wl)[e % E, c*128 + p]."""
    vh = wh.T.reshape(N_CH, 128, E)
    vl = wl.T.reshape(N_CH, 128, E)
    v = np.concatenate([vh, vl], axis=2)          # [N_CH, 128, 128]
    return np.ascontiguousarray(v.transpose(1, 0, 2)).reshape(128, N_CH * 128)


def pack_w8(w):
    """w [E, H] f32 -> e4m3 [128, N_CH*E]: out[p, c*E + e] = w8[e, c*128+p]."""
    w8 = (w * 2.0 ** WB).astype(ml_dtypes.float8_e4m3)
    v = w8.T.reshape(N_CH, 128, E)
    return np.ascontiguousarray(v.transpose(1, 0, 2)).reshape(128, N_CH * E)


_NC_CACHE = {}


def host_pack(hidden_states, expert_bias, gate_w):
    x2 = np.asarray(hidden_states, dtype=np.float32).reshape(T_FULL, H)
    w = np.asarray(gate_w, dtype=np.float32)
    bias = np.asarray(expert_bias, dtype=np.float32)

    xh0 = x2.astype(np.float16)
    r = (x2 - xh0.astype(np.float32)) * float(2.0 ** XA)
    xh = (xh0.astype(np.float32) * float(2.0 ** XS)).astype(np.float16)
    ws = float(2.0 ** WS)
    wh = (w.astype(np.float16).astype(np.float32) * ws).astype(np.float16)
    wl = ((w - w.astype(np.float16).astype(np.float32)) * ws).astype(np.float16)

    w2_p = pack_w2(wh, wl)
    w8_p = pack_w8(w)
    bias_p = np.ascontiguousarray(np.broadcast_to(bias[None, :], (128, E)))
    jab = np.zeros((128, E), dtype=np.float32)
    jab[0:E, :] = np.eye(E) * CSCALE
    jab[E:2 * E, :] = np.eye(E) * CSCALE
    jab_p = jab

    in_maps = []
    for k in range(N_CORES):
        rows = slice(k * T_CORE, (k + 1) * T_CORE)
        in_maps.append({
            "xh_d": pack_x_shard(np.ascontiguousarray(xh[rows].T), np.float16),
            "xl_d": pack_x_shard(np.ascontiguousarray(r[rows].T.astype(np.float32)),
                                 ml_dtypes.float8_e4m3),
            "w2_d": w2_p,
            "w8_d": w8_p,
            "bias_d": bias_p,
            "jab_d": jab_p,
        })
    return in_maps


def kernel(hidden_states, expert_bias, gate_w):
    in_maps = host_pack(hidden_states, expert_bias, gate_w)

    if "nc" not in _NC_CACHE:
        _NC_CACHE["nc"] = build_nc()
    nc = _NC_CACHE["nc"]

    res = bass_utils.run_bass_kernel_spmd(nc, in_maps, core_ids=list(range(N_CORES)))

    scores = np.empty((T_FULL, TOPK), dtype=np.float32)
    idx = np.empty((T_FULL, TOPK), dtype=np.int32)
    for k in range(N_CORES):
        o = res.results[k]
        s = o["out_s_d"].reshape(128, N_TILES, TOPK).transpose(1, 0, 2).reshape(T_CORE, TOPK)
        i = o["out_i_d"].view(np.int32).reshape(128, N_TILES, TOPK).transpose(1, 0, 2).reshape(T_CORE, TOPK)
        scores[k * T_CORE:(k + 1) * T_CORE] = s
        idx[k * T_CORE:(k + 1) * T_CORE] = i
    return scores, idx



# revision 37
# speedup vs baseline: 1.0532x; 1.0532x over previous
"""AfmoeTokenChoiceRouter kernel for 8x Trainium2 NeuronCores.

Data-parallel over tokens: each of the 8 cores handles 2048 tokens.

Precision scheme (3 bytes/element of x instead of 4):
  x  = xh (fp16) + r,   r shipped as xl8 = e4m3(r * 2^16)     [1 byte]
  w  = wh (fp16) + wl (fp16)  [replicated, tiny]
  w8 = e4m3(w * 2^11)   [replicated, tiny]
The fp16 stream is shipped pre-scaled (xh*2^13, w*2^14 -- exact exponent
shifts) so its products land at x*w*2^27, the SAME scale as the fp8 stream's
(r*2^16)*(w*2^11): both streams accumulate into ONE PSUM region and the
2^-27 folds into the combine constant. Top-8 selection matches a pure-fp32
reference on all but ~4 near-tie tokens in 16384 (L2 idx rel err ~5e-3),
while HBM traffic drops from 16.8 MB to 12.6 MB per core.

Per core pipeline (supertile blocks per SCHEDULE, host-packed so every DMA
is a contiguous full-rate burst; tapering tail keeps the serial drain short):
  - DMA: xh (fp16) on the sync HWDGE queue, xl8 (fp8) on the ACT queue
  - PE per block: 16 fp16 matmuls ([wh_c|wl_c] stationary x xh_c -> psum
    rows 0:64 wh terms + 2^-13-aligned, 64:128 wl terms), then 8 fp8
    DoubleRow matmuls (2 k-chunks each, w8 pairs stationary) accumulating
    the residual into rows 0:64 of the same psum. Grouping the fp8 stream
    after the fp16 stream (one dtype/stationary switch per block) measured
    ~1.6x faster than interleaving them per-chunk on hardware.
  - PE per 128-token tile: one "J-matmul" (data block stationary, constant
    jA = 2^-27*[I64;I64] moving) fuses the back-transpose, the hi+lo row
    fold and the 2^-27 descale in a single instruction -> l_ps [tok, 64]
  - the J-matmuls + topk of block i are emitted AFTER block i+1's GEMM
    matmuls (_DEFER_J): the PE engine queue is in-order, so without the
    deferral each block's J-matmuls stall the PE behind the ACT
    PSUM->SBUF copy round-trip instead of running block i+1's matmuls
  - ACT sigmoid; DVE top-8: max8/max_index on biased scores, per-tile
    threshold mask (TimelineSim: batching the mask across tiles adds a
    cross-tile barrier and is a net loss), second max8 pass on masked
    unbiased scores, 8x8 index-match reorder directly on u32 indices
    (saves 2 copies/block; TimelineSim −0.7us), normalize, scale 2.5
  - outputs stored via the ACT HWDGE queue (15 head tiles early, 1-tile
    tail at the end, scores/indices tails split across the ACT/SP rings
    so their ~2us HBM write-completion latencies overlap); SWDGE
    descriptor generation (~6us per strided store) and per-block store
    dribble both measured as tail serializers. TimelineSim shows the
    drain (last 128-tok block's DMA->PE->copy->J->sigmoid->topk->store
    chain) is ~6.6us and is the main residual overhead beyond the DMA
    window; schedule taper + early stores exist to minimize it.
Outputs per core: scores [128, 16, 8] f32 and indices [128, 16, 8] u32 in
partition-major token order (token = 128*tile + partition), unpermuted on
the host.
"""

import contextlib as _contextlib

import numpy as np
import ml_dtypes

import concourse.bass as bass
import concourse.mybir as mybir
import concourse.tile as tile
import concourse.bass_utils as bass_utils
from concourse import bacc
from concourse.masks import make_identity

f32 = mybir.dt.float32
f16 = mybir.dt.float16
f8 = mybir.dt.float8e4
u32 = mybir.dt.uint32
Alu = mybir.AluOpType
Act = mybir.ActivationFunctionType
_nullctx = _contextlib.nullcontext

N_CORES = 8
T_FULL, H, E, TOPK = 16384, 2048, 64, 8
T_CORE = T_FULL // N_CORES          # 2048
N_TILES = T_CORE // 128             # 16
N_CH = H // 128                     # 16 contraction chunks
ROUTE_SCALE = 2.5
# supertile schedule; baked into the host packing (each block is stored
# contiguously so every DMA is a full-rate linear burst). Tapering tail keeps
# the post-last-DMA serial chain short.
SCHEDULE = [384, 384, 384, 384, 256, 128, 128]
XA = 16                             # xl8 = e4m3(r * 2^XA), max |val| = 128
WB = 11                             # w8 = e4m3(w * 2^WB), max |val| = 224
# the fp16 stream is shipped pre-scaled (xh*2^13, w*2^14) so its PSUM terms
# land at x*w*2^27 == the fp8 stream's r*2^16 * w*2^11 scale: both streams
# accumulate into ONE psum, and the 2^-27 folds into the combine constant.
XS, WS = 13, 14
CSCALE = 2.0 ** (-(XA + WB))
assert XS + WS == XA + WB


def router_body(tc, outs, ins, reps=1, skip_dma=False, skip_compute=False,
                n_terms=3, skip_topk=False):
    nc = tc.nc
    out_s_d, out_i_d = outs
    xh_d, xl_d, w2_d, w8_d, bias_d, jab_d = ins

    with (
        tc.tile_pool(name="const", bufs=1) as constp,
        tc.tile_pool(name="xin", bufs=globals().get('_XBUFS', 3)) as xpool,
        tc.tile_pool(name="persist", bufs=1) as pers,
        tc.tile_pool(name="scratch", bufs=globals().get('_SCRBUFS', 4)) as scr,
        tc.tile_pool(name="ps_lt", bufs=globals().get('_LTBUFS', 3), space="PSUM") as ps_lt,
        tc.tile_pool(name="ps_l", bufs=globals().get('_PLBUFS', 3), space="PSUM") as ps_l,
    ):
        # setup DMAs ride the ACT HWDGE queue so they don't delay the first
        # x pieces on the sync queue (HWDGE is FIFO per issuing engine).
        # w2 is split into pieces so the first chunks' matmuls can start
        # before the whole 512 KB stationary lands; bias/jab (needed only
        # ~8us in) are deferred until after the first xl block (see below).
        w2_sb = constp.tile([128, N_CH, 128], f16)
        w8_sb = constp.tile([128, N_CH, E], f8)
        w2_r = w2_d.rearrange("p (c e) -> p c e", e=128)
        wdef = globals().get('_W2_DEFER', 0)
        late_w = []
        for d0 in range(0, N_CH, 4):
            if wdef and d0 >= wdef:
                late_w.append((w2_sb[:, d0:d0 + 4, :], w2_r[:, d0:d0 + 4, :]))
            else:
                nc.scalar.dma_start(w2_sb[:, d0:d0 + 4, :], w2_r[:, d0:d0 + 4, :])
        if wdef:
            late_w.append((w8_sb[:], w8_d))
        else:
            nc.scalar.dma_start(w8_sb[:], w8_d)
        bias_sb = constp.tile([128, 1, E], f32)
        # combine constants: jA = [I64; I64], jB = 2^-27 * I64. Used as the
        # moving operand of per-tile "transpose" matmuls that fuse the
        # back-transpose with the hi+lo add and the scaled fp8-term add.
        jab_sb = constp.tile([128, 1, E], f32)
        setup_rest = late_w + [(bias_sb, bias_d), (jab_sb, jab_d)]
        if skip_dma:
            while setup_rest:
                sb, dr = setup_rest.pop(0)
                nc.scalar.dma_start(sb[:], dr)

        # persistent per-core tensors
        s_all = pers.tile([128, N_TILES, E], f32)      # sigmoid scores
        b_all = pers.tile([128, N_TILES, E], f32)      # biased scores
        vb_all = pers.tile([128, N_TILES, 8], f32)     # top8 of biased
        vs_all = pers.tile([128, N_TILES, 8], f32)     # top8 of masked s
        ib_all = pers.tile([128, N_TILES, 8], u32)     # indices (biased order)
        is_all = pers.tile([128, N_TILES, 8], u32)     # indices (s order)
        ibf = pers.tile([128, N_TILES, 8], f32)
        isf = pers.tile([128, N_TILES, 8], f32)
        out_s_sb = pers.tile([128, N_TILES, 8], f32)

        DMA_CH = globals().get('_DMA_CH_OVERRIDE', 4)   # h-chunks per xh piece
        DMA_CHL = globals().get('_DMA_CHL_OVERRIDE', 16)  # h-chunks per xl piece

        def supertile(pos, tok_st):
            tiles_ss = tok_st // 128
            t0 = pos // 128
            s4 = slice(t0, t0 + tiles_ss)
            xh_sb = xpool.tile([128, N_CH, tok_st], f16, tag="xh")
            xl_sb = xpool.tile([128, N_CH, tok_st], f8, tag="xl")
            foff = N_CH * pos
            xh_st = xh_d[:, foff:foff + N_CH * tok_st].rearrange(
                "p (c t) -> p c t", t=tok_st)
            xl_st = xl_d[:, foff:foff + N_CH * tok_st].rearrange(
                "p (c t) -> p c t", t=tok_st)
            if not skip_dma:
                bal = globals().get('_BAL_QUEUES', 0)
                xl_eng = nc.scalar if globals().get('_XL_ON_ACT', 1) else nc.sync
                if bal == 3:
                    # 3-ring split: xh pieces round-robin sync/ACT/pool-SWDGE,
                    # xl pieces round-robin the same three
                    engs = [nc.sync, nc.scalar, nc.gpsimd]
                    d0 = 0
                    pi = 0
                    while d0 < N_CH:
                        dn = min(DMA_CH, N_CH - d0)
                        engs[pi % 3].dma_start(
                            xh_sb[:, d0:d0 + dn, :], xh_st[:, d0:d0 + dn, :])
                        d0 += dn
                        pi += 1
                    d0 = 0
                    while d0 < N_CH:
                        dn = min(DMA_CHL // 2, N_CH - d0)
                        engs[pi % 3].dma_start(
                            xl_sb[:, d0:d0 + dn, :], xl_st[:, d0:d0 + dn, :])
                        d0 += dn
                        pi += 1
                else:
                    d0 = 0
                    pi = 0
                    while d0 < N_CH:
                        dn = min(DMA_CH, N_CH - d0)
                        eng = nc.sync if (not bal or pi % 2 == 0) else nc.scalar
                        eng.dma_start(xh_sb[:, d0:d0 + dn, :], xh_st[:, d0:d0 + dn, :])
                        d0 += dn
                        pi += 1
                    d0 = 0
                    pi = 0
                    while d0 < N_CH:
                        dn = min(DMA_CHL, N_CH - d0)
                        eng = xl_eng if (not bal or pi % 2 == 0) else nc.sync
                        eng.dma_start(xl_sb[:, d0:d0 + dn, :], xl_st[:, d0:d0 + dn, :])
                        d0 += dn
                        pi += 1
                while setup_rest:
                    sb, dr = setup_rest.pop(0)
                    nc.scalar.dma_start(sb[:], dr)
            else:
                # timing-ablation mode: mark x tiles written so the tile
                # framework doesn't assert on read-without-write
                nc.gpsimd.memset(xh_sb[:, 0, :1], 0)
                nc.gpsimd.memset(xl_sb[:, 0, :1], 0)
            if skip_compute:
                return

            # GEMM: psumA <- [wh_c|wl_c] fp16 x xh_c (rows 0:64 wh, 64:128 wl)
            #       psumB <- w8_c fp8 x xl8_c (rows 0:64)
            # both streams accumulate into one psum: fp16 terms into rows
            # 0:128 ([wh|wl] stationary), fp8 residual terms into rows 0:64
            # (same scale 2^27 by construction)
            lt_ps = ps_lt.tile([128, tok_st], f32, tag="lt")
            use_dr = globals().get('_DR', 1) and \
                tok_st >= globals().get('_DR_MIN_FD', 0)
            b_sep = globals().get('_BSEP', 1)
            for c in range(N_CH):
                last_a = c == N_CH - 1 and n_terms < 3
                nc.tensor.matmul(lt_ps[:], w2_sb[:, c, :], xh_sb[:, c, :],
                                 start=(c == 0), stop=last_a)
                if n_terms >= 3 and not use_dr and not b_sep:
                    nc.tensor.matmul(lt_ps[0:64, :], w8_sb[:, c, :],
                                     xl_sb[:, c, :],
                                     start=False, stop=(c == N_CH - 1))
            if n_terms >= 3 and not use_dr and b_sep:
                for c in range(N_CH):
                    nc.tensor.matmul(lt_ps[0:64, :], w8_sb[:, c, :],
                                     xl_sb[:, c, :],
                                     start=False, stop=(c == N_CH - 1))
            if n_terms >= 3 and use_dr:
                # fp8 DoubleRow: 2 contraction chunks per matmul
                # (out = sum_j lhsT[:, j, :].T @ rhs[:, j, :])
                for cc in range(N_CH // 2):
                    nc.tensor.matmul(
                        lt_ps[0:64, :], w8_sb[:, 2 * cc:2 * cc + 2, :],
                        xl_sb[:, 2 * cc:2 * cc + 2, :],
                        start=False, stop=(cc == N_CH // 2 - 1),
                        perf_mode=mybir.MatmulPerfMode.DoubleRow)
            if n_terms < 3:
                dummy = scr.tile([128, 1], f8, tag="dummy")
                nc.vector.tensor_copy(dummy[:], xl_sb[:, 0, :1])

            if globals().get('_DEFER_COPY', 0):
                return lt_ps
            return do_copy(tok_st, lt_ps)

        def do_copy(tok_st, lt_ps):
            lt_sb = scr.tile([128, tok_st], f32, tag="ltsb")
            cp = globals().get('_COPY_ENG', 'scalar')
            if cp == 'pool':
                nc.gpsimd.tensor_copy(lt_sb[:], lt_ps[:])
            elif cp == 'vector':
                nc.vector.tensor_copy(lt_sb[:], lt_ps[:])
            else:
                nc.scalar.copy(lt_sb[:], lt_ps[:])
            return lt_sb

        def finish_block(pos, tok_st, lt_sb, last=None):
            tiles_ss = tok_st // 128
            t0 = pos // 128
            s4 = slice(t0, t0 + tiles_ss)
            # fused back-transpose + combine: per 128-token block,
            #   l_ps[t, e] = sum_r lt[r, t] * jA[r, e] = 2^-27 * (hi + lo rows)
            # (the data block is the stationary, jA the 64-col moving)
            l_ps = ps_l.tile([128, tiles_ss, E], f32, tag="lps")
            for q in range(tiles_ss):
                qs = slice(q * 128, (q + 1) * 128)
                nc.tensor.matmul(l_ps[:, q, :], lt_sb[:, qs], jab_sb[:, 0, :],
                                 start=True, stop=True)

            s_sl = s_all[:, s4, :]
            nc.scalar.activation(s_sl, l_ps[:, :, :], Act.Sigmoid)
            if skip_topk:
                nc.vector.tensor_copy(out_s_sb[:, s4, :], s_sl[:, :, :8])
                nc.vector.tensor_copy(ib_all[:, s4, :], s_sl[:, :, 8:16])
                return
            b_sl = b_all[:, s4, :]
            beng = nc.gpsimd if globals().get('_BIAS_ON_POOL', 0) else nc.vector
            beng.tensor_tensor(
                out=b_sl, in0=s_sl,
                in1=bias_sb[:].broadcast_to([128, tiles_ss, E]),
                op=Alu.add,
            )

            for q in range(tiles_ss):
                i = t0 + q
                nc.vector.max(out=vb_all[:, i, :], in_=b_all[:, i, :])
                nc.vector.max_index(out=ib_all[:, i, :], in_max=vb_all[:, i, :],
                                    in_values=b_all[:, i, :])
            if last is not None:
                # the indices tail store needs only max_index output; issuing
                # it here lets its ~2us HBM write receipt overlap the rest of
                # the score chain (mask/2nd pass/normalize) of the last block
                od_i = out_i_d.rearrange("p (i k) -> p i k", k=8)
                nc.sync.dma_start(od_i[:, last:, :], ib_all[:, last:, :])

            # selected-expert masking: sarr = (b >= thr8) * s
            sarr = scr.tile([128, tiles_ss, E], f32, tag="sarr")
            if globals().get('_BATCH_MASK', 0):
                ge = scr.tile([128, tiles_ss, E], f32, tag="ge")
                nc.vector.tensor_tensor(
                    out=ge[:], in0=b_all[:, s4, :],
                    in1=vb_all[:, s4, 7:8].broadcast_to([128, tiles_ss, E]),
                    op=Alu.is_ge)
                nc.vector.tensor_tensor(
                    out=sarr[:], in0=ge[:], in1=s_all[:, s4, :], op=Alu.mult)
            else:
                for q in range(tiles_ss):
                    i = t0 + q
                    nc.vector.scalar_tensor_tensor(
                        out=sarr[:, q, :], in0=b_all[:, i, :],
                        scalar=vb_all[:, i, 7:8], in1=s_all[:, i, :],
                        op0=Alu.is_ge, op1=Alu.mult)

            for q in range(tiles_ss):
                i = t0 + q
                nc.vector.max(out=vs_all[:, i, :], in_=sarr[:, q, :])
                nc.vector.max_index(out=is_all[:, i, :], in_max=vs_all[:, i, :],
                                    in_values=sarr[:, q, :])

            # reorder vs_all (s-descending) into biased-rank order by idx match
            eeng = nc.gpsimd if globals().get('_EQ_ON_POOL', 0) else nc.vector
            eq = scr.tile([128, tiles_ss, 8, 8], f32, tag="eq")
            if globals().get('_EQ_U32', 1):
                eeng.tensor_tensor(
                    out=eq[:],
                    in0=ib_all[:, s4, :].broadcast_to([128, tiles_ss, 8, 8]),
                    in1=is_all[:, s4, :][:, :, None, :].broadcast_to(
                        [128, tiles_ss, 8, 8]),
                    op=Alu.is_equal,
                )
            else:
                nc.vector.tensor_copy(ibf[:, s4, :], ib_all[:, s4, :])
                nc.vector.tensor_copy(isf[:, s4, :], is_all[:, s4, :])
                eeng.tensor_tensor(
                    out=eq[:],
                    in0=ibf[:, s4, :].broadcast_to([128, tiles_ss, 8, 8]),
                    in1=isf[:, s4, :][:, :, None, :].broadcast_to(
                        [128, tiles_ss, 8, 8]),
                    op=Alu.is_equal,
                )
            g_sc = scr.tile([128, tiles_ss, 8, 8], f32, tag="g")
            eeng.tensor_tensor(
                out=g_sc[:], in0=eq[:],
                in1=vs_all[:, s4, :][:, :, None, :].broadcast_to(
                    [128, tiles_ss, 8, 8]),
                op=Alu.mult,
            )
            tsr = scr.tile([128, tiles_ss, 8], f32, tag="tsr")
            nc.vector.reduce_sum(out=tsr[:], in_=g_sc[:], axis=mybir.AxisListType.X)

            den = scr.tile([128, tiles_ss], f32, tag="den")
            nc.vector.reduce_sum(out=den[:], in_=vs_all[:, s4, :],
                                 axis=mybir.AxisListType.X)
            rec = scr.tile([128, tiles_ss], f32, tag="rec")
            nc.vector.reciprocal(rec[:], den[:])
            nc.vector.scalar_tensor_tensor(
                out=out_s_sb[:, s4, :], in0=tsr[:], scalar=ROUTE_SCALE,
                in1=rec[:].broadcast_to([128, tiles_ss, 8]),
                op0=Alu.mult, op1=Alu.mult,
            )
            if globals().get('_OUT_PER_ST', 0):
                od_s = out_s_d.rearrange("p (i k) -> p i k", k=8)
                od_i = out_i_d.rearrange("p (i k) -> p i k", k=8)
                nc.scalar.dma_start(od_s[:, s4, :], out_s_sb[:, s4, :])
                nc.scalar.dma_start(od_i[:, s4, :], ib_all[:, s4, :])

        schedule = globals().get('_SCHED', SCHEDULE)
        assert sum(schedule) == T_CORE

        def whole_pass():
            pos = 0
            pos_fin = 0
            tail0 = 0
            n_early = globals().get('_EARLY_TILES', 15)
            hi_last = globals().get('_HI_LAST', 0)
            defer = globals().get('_DEFER_J', 1)
            early_done = False
            pending = None          # (pos, tok_st, lt_sb) not yet finished
            store_out = not skip_compute and not skip_topk and \
                not globals().get('_OUT_PER_ST', 0)

            def maybe_early_store():
                nonlocal early_done, tail0
                if store_out and not early_done and pos_fin >= 128 * n_early:
                    # store the finished head tiles while the x stream still
                    # runs; only the short tail rides the final store pair
                    e = pos_fin // 128
                    od_s = out_s_d.rearrange("p (i k) -> p i k", k=8)
                    od_i = out_i_d.rearrange("p (i k) -> p i k", k=8)
                    nc.scalar.dma_start(od_s[:, 0:e, :], out_s_sb[:, 0:e, :])
                    nc.scalar.dma_start(od_i[:, 0:e, :], ib_all[:, 0:e, :])
                    early_done = True
                    tail0 = e

            warm = globals().get('_PE_WARM', 0)
            if warm and not skip_compute:
                # HAM keep-warm filler: PE idles ~5us at each iteration start
                # (all-engine loop barrier + first block's DMA), long enough
                # for the clock gate to re-throttle to 1.2 GHz. Issue dummy
                # matmuls with no DMA dependency to span the gap and hold the
                # 2.4 GHz clock. Sized to finish before the first block lands.
                wm_ps = ps_lt.tile([128, 128], f32, tag="lt")
                for _ in range(warm):
                    nc.tensor.matmul(wm_ps[:], w2_sb[:, 0, :], w2_sb[:, 0, :],
                                     start=True, stop=True)

            for bi, tok_st in enumerate(schedule):
                hp = hi_last and bi >= len(schedule) - hi_last
                with tc.high_priority() if hp else _nullctx():
                    lt_sb = supertile(pos, tok_st)
                if not skip_compute:
                    if defer:
                        if pending is not None:
                            p_pos, p_tok, p_lt = pending
                            if globals().get('_DEFER_COPY', 0):
                                p_lt = do_copy(p_tok, p_lt)
                            finish_block(p_pos, p_tok, p_lt)
                            pos_fin = p_pos + p_tok
                        pending = (pos, tok_st, lt_sb)
                    else:
                        finish_block(pos, tok_st, lt_sb)
                        pos_fin = pos + tok_st
                pos += tok_st
                maybe_early_store()
            idx_tail_done = None
            if pending is not None:
                p_pos, p_tok, p_lt = pending
                # if the stored tail lies entirely within the last block,
                # fire its indices store early (right after max_index)
                if store_out and globals().get('_EARLY_IDX_TAIL', 1) \
                        and n_early >= p_pos // 128:
                    idx_tail_done = max(p_pos // 128, n_early)
                with tc.high_priority() if globals().get('_HI_FLUSH', 0) \
                        else _nullctx():
                    if globals().get('_DEFER_COPY', 0):
                        p_lt = do_copy(p_tok, p_lt)
                    finish_block(p_pos, p_tok, p_lt, last=idx_tail_done)
                pos_fin = p_pos + p_tok
                maybe_early_store()
            if not skip_compute and not skip_topk and not globals().get('_OUT_PER_ST', 0):
                t0 = tail0 if early_done else 0
                od_s = out_s_d.rearrange("p (i k) -> p i k", k=8)
                od_i = out_i_d.rearrange("p (i k) -> p i k", k=8)
                ieng = nc.sync if globals().get('_STORE_SPLIT', 1) else nc.scalar
                nc.scalar.dma_start(od_s[:, t0:, :], out_s_sb[:, t0:, :])
                # indices tail may already be (partially) stored by the last
                # finish_block's early-idx store, which covered [ie, N_TILES)
                ie = idx_tail_done if idx_tail_done is not None else N_TILES
                if ie > t0:
                    ieng.dma_start(od_i[:, t0:ie, :], ib_all[:, t0:ie, :])

        if reps == 1:
            whole_pass()
        else:
            with tc.For_i(0, reps, 1,
                          staggered_reset=bool(globals().get('_STAGGER', 0))):
                whole_pass()


def build_nc(reps=1, skip_dma=False, skip_compute=False, n_terms=3, skip_topk=False):
    nc = bacc.Bacc("TRN2", target_bir_lowering=False, debug=False)
    xh_d = nc.dram_tensor("xh_d", [128, N_CH * T_CORE], f16, kind="ExternalInput")
    xl_d = nc.dram_tensor("xl_d", [128, N_CH * T_CORE], f8, kind="ExternalInput")
    w2_d = nc.dram_tensor("w2_d", [128, N_CH * 128], f16, kind="ExternalInput")
    w8_d = nc.dram_tensor("w8_d", [128, N_CH * E], f8, kind="ExternalInput")
    bias_d = nc.dram_tensor("bias_d", [128, E], f32, kind="ExternalInput")
    jab_d = nc.dram_tensor("jab_d", [128, E], f32, kind="ExternalInput")
    out_s_d = nc.dram_tensor("out_s_d", [128, N_TILES * 8], f32, kind="ExternalOutput")
    out_i_d = nc.dram_tensor("out_i_d", [128, N_TILES * 8], u32, kind="ExternalOutput")

    with tile.TileContext(nc) as tc:
        router_body(
            tc,
            (out_s_d.ap(), out_i_d.ap()),
            (xh_d.ap(), xl_d.ap(), w2_d.ap(), w8_d.ap(), bias_d.ap(), jab_d.ap()),
            reps=reps, skip_dma=skip_dma, skip_compute=skip_compute,
            n_terms=n_terms, skip_topk=skip_topk,
        )
    nc.compile()
    return nc


def pack_x_shard(xT, dtype):
    """[H, T_CORE] -> [128, N_CH*T_CORE] with each SCHEDULE block stored
    contiguously: out[p, N_CH*pos + c*tok_st + t] = xT[c*128 + p, pos + t]."""
    v = xT.reshape(N_CH, 128, T_CORE)
    blocks = []
    pos = 0
    for tok_st in globals().get('_SCHED', SCHEDULE):
        blk = v[:, :, pos:pos + tok_st]            # [N_CH, 128, tok_st]
        blocks.append(blk.transpose(1, 0, 2).reshape(128, N_CH * tok_st))
        pos += tok_st
    return np.ascontiguousarray(np.concatenate(blocks, axis=1)).astype(dtype)


def pack_w2(wh, wl):
    """wh/wl [E, H] fp16 -> [128, N_CH*128] with wh in cols 0:64, wl in 64:128
    of each chunk: out[p, c*128 + e] = (wh if e < E else wl)[e % E, c*128 + p]."""
    vh = wh.T.reshape(N_CH, 128, E)
    vl = wl.T.reshape(N_CH, 128, E)
    v = np.concatenate([vh, vl], axis=2)          # [N_CH, 128, 128]
    return np.ascontiguousarray(v.transpose(1, 0, 2)).reshape(128, N_CH * 128)


def pack_w8(w):
    """w [E, H] f32 -> e4m3 [128, N_CH*E]: out[p, c*E + e] = w8[e, c*128+p]."""
    w8 = (w * 2.0 ** WB).astype(ml_dtypes.float8_e4m3)
    v = w8.T.reshape(N_CH, 128, E)
    return np.ascontiguousarray(v.transpose(1, 0, 2)).reshape(128, N_CH * E)


_NC_CACHE = {}


def host_pack(hidden_states, expert_bias, gate_w):
    x2 = np.asarray(hidden_states, dtype=np.float32).reshape(T_FULL, H)
    w = np.asarray(gate_w, dtype=np.float32)
    bias = np.asarray(expert_bias, dtype=np.float32)

    xh0 = x2.astype(np.float16)
    r = (x2 - xh0.astype(np.float32)) * float(2.0 ** XA)
    xh = (xh0.astype(np.float32) * float(2.0 ** XS)).astype(np.float16)
    ws = float(2.0 ** WS)
    wh = (w.astype(np.float16).astype(np.float32) * ws).astype(np.float16)
    wl = ((w - w.astype(np.float16).astype(np.float32)) * ws).astype(np.float16)

    w2_p = pack_w2(wh, wl)
    w8_p = pack_w8(w)
    bias_p = np.ascontiguousarray(np.broadcast_to(bias[None, :], (128, E)))
    jab = np.zeros((128, E), dtype=np.float32)
    jab[0:E, :] = np.eye(E) * CSCALE
    jab[E:2 * E, :] = np.eye(E) * CSCALE
    jab_p = jab

    in_maps = []
    for k in range(N_CORES):
        rows = slice(k * T_CORE, (k + 1) * T_CORE)
        in_maps.append({
            "xh_d": pack_x_shard(np.ascontiguousarray(xh[rows].T), np.float16),
            "xl_d": pack_x_shard(np.ascontiguousarray(r[rows].T.astype(np.float32)),
                                 ml_dtypes.float8_e4m3),
            "w2_d": w2_p,
            "w8_d": w8_p,
            "bias_d": bias_p,
            "jab_d": jab_p,
        })
    return in_maps


def kernel(hidden_states, expert_bias, gate_w):
    in_maps = host_pack(hidden_states, expert_bias, gate_w)

    if "nc" not in _NC_CACHE:
        _NC_CACHE["nc"] = build_nc()
    nc = _NC_CACHE["nc"]

    res = bass_utils.run_bass_kernel_spmd(nc, in_maps, core_ids=list(range(N_CORES)))

    scores = np.empty((T_FULL, TOPK), dtype=np.float32)
    idx = np.empty((T_FULL, TOPK), dtype=np.int32)
    for k in range(N_CORES):
        o = res.results[k]
        s = o["out_s_d"].reshape(128, N_TILES, TOPK).transpose(1, 0, 2).reshape(T_CORE, TOPK)
        i = o["out_i_d"].view(np.int32).reshape(128, N_TILES, TOPK).transpose(1, 0, 2).reshape(T_CORE, TOPK)
        scores[k * T_CORE:(k + 1) * T_CORE] = s
        idx[k * T_CORE:(k + 1) * T_CORE] = i
    return scores, idx



# revision 39
# speedup vs baseline: 1.0870x; 1.0321x over previous
"""AfmoeTokenChoiceRouter kernel for 8x Trainium2 NeuronCores.

Data-parallel over tokens: each of the 8 cores handles 2048 tokens.

Precision scheme (3 bytes/element of x instead of 4):
  x  = xh (fp16) + r,   r shipped as xl8 = e4m3(r * 2^16)     [1 byte]
  w  = wh (fp16) + wl (fp16)  [replicated, tiny]
  w8 = e4m3(w * 2^11)   [replicated, tiny]
The fp16 stream is shipped pre-scaled (xh*2^13, w*2^14 -- exact exponent
shifts) so its products land at x*w*2^27, the SAME scale as the fp8 stream's
(r*2^16)*(w*2^11): both streams accumulate into ONE PSUM region and the
2^-27 folds into the combine constant. Top-8 selection matches a pure-fp32
reference on all but ~4 near-tie tokens in 16384 (L2 idx rel err ~5e-3),
while HBM traffic drops from 16.8 MB to 12.6 MB per core.

Per core pipeline (supertile blocks per SCHEDULE, host-packed so every DMA
is a contiguous full-rate burst; tapering tail keeps the serial drain short):
  - DMA: xh (fp16) on the sync HWDGE queue, xl8 (fp8) on the ACT queue
  - PE per block: 16 fp16 matmuls ([wh_c|wl_c] stationary x xh_c -> psum
    rows 0:64 wh terms + 2^-13-aligned, 64:128 wl terms), then 8 fp8
    DoubleRow matmuls (2 k-chunks each, w8 pairs stationary) accumulating
    the residual into rows 0:64 of the same psum. Grouping the fp8 stream
    after the fp16 stream (one dtype/stationary switch per block) measured
    ~1.6x faster than interleaving them per-chunk on hardware.
  - PE per 128-token tile: one "J-matmul" (data block stationary, constant
    jA = 2^-27*[I64;I64] moving) fuses the back-transpose, the hi+lo row
    fold and the 2^-27 descale in a single instruction -> l_ps [tok, 64]
  - the J-matmuls + topk of block i are emitted AFTER block i+1's GEMM
    matmuls (_DEFER_J): the PE engine queue is in-order, so without the
    deferral each block's J-matmuls stall the PE behind the ACT
    PSUM->SBUF copy round-trip instead of running block i+1's matmuls
  - ACT sigmoid; DVE top-8: max8/max_index on biased scores, per-tile
    threshold mask (TimelineSim: batching the mask across tiles adds a
    cross-tile barrier and is a net loss), second max8 pass on masked
    unbiased scores, 8x8 index-match reorder directly on u32 indices
    (saves 2 copies/block; TimelineSim −0.7us), normalize, scale 2.5
  - outputs stored via the ACT HWDGE queue (15 head tiles early, 1-tile
    tail at the end, scores/indices tails split across the ACT/SP rings
    so their ~2us HBM write-completion latencies overlap); SWDGE
    descriptor generation (~6us per strided store) and per-block store
    dribble both measured as tail serializers. TimelineSim shows the
    drain (last 128-tok block's DMA->PE->copy->J->sigmoid->topk->store
    chain) is ~6.6us and is the main residual overhead beyond the DMA
    window; schedule taper + early stores exist to minimize it.
Outputs per core: scores [128, 16, 8] f32 and indices [128, 16, 8] u32 in
partition-major token order (token = 128*tile + partition), unpermuted on
the host.
"""

import contextlib as _contextlib

import numpy as np
import ml_dtypes

import concourse.bass as bass
import concourse.mybir as mybir
import concourse.tile as tile
import concourse.bass_utils as bass_utils
from concourse import bacc
from concourse.masks import make_identity

f32 = mybir.dt.float32
f16 = mybir.dt.float16
f8 = mybir.dt.float8e4
u32 = mybir.dt.uint32
Alu = mybir.AluOpType
Act = mybir.ActivationFunctionType
_nullctx = _contextlib.nullcontext

N_CORES = 8
T_FULL, H, E, TOPK = 16384, 2048, 64, 8
T_CORE = T_FULL // N_CORES          # 2048
N_TILES = T_CORE // 128             # 16
N_CH = H // 128                     # 16 contraction chunks
ROUTE_SCALE = 2.5
# supertile schedule; baked into the host packing (each block is stored
# contiguously so every DMA is a full-rate linear burst). Tapering tail keeps
# the post-last-DMA serial chain short.
SCHEDULE = [384, 384, 384, 384, 256, 128, 128]
XA = 16                             # xl8 = e4m3(r * 2^XA), max |val| = 128
WB = 11                             # w8 = e4m3(w * 2^WB), max |val| = 224
# the fp16 stream is shipped pre-scaled (xh*2^13, w*2^14) so its PSUM terms
# land at x*w*2^27 == the fp8 stream's r*2^16 * w*2^11 scale: both streams
# accumulate into ONE psum, and the 2^-27 folds into the combine constant.
XS, WS = 13, 14
CSCALE = 2.0 ** (-(XA + WB))
assert XS + WS == XA + WB


def router_body(tc, outs, ins, reps=1, skip_dma=False, skip_compute=False,
                n_terms=3, skip_topk=False):
    nc = tc.nc
    out_s_d, out_i_d = outs
    xh_d, xl_d, w2_d, w8_d, bias_d, jab_d = ins

    with (
        tc.tile_pool(name="const", bufs=1) as constp,
        tc.tile_pool(name="xin", bufs=globals().get('_XBUFS', 3)) as xpool,
        tc.tile_pool(name="persist", bufs=1) as pers,
        tc.tile_pool(name="scratch", bufs=globals().get('_SCRBUFS', 4)) as scr,
        tc.tile_pool(name="ps_lt", bufs=globals().get('_LTBUFS', 3), space="PSUM") as ps_lt,
        tc.tile_pool(name="ps_l", bufs=globals().get('_PLBUFS', 3), space="PSUM") as ps_l,
    ):
        # setup DMAs ride the ACT HWDGE queue so they don't delay the first
        # x pieces on the sync queue (HWDGE is FIFO per issuing engine).
        # w2 is split into pieces so the first chunks' matmuls can start
        # before the whole 512 KB stationary lands; bias/jab (needed only
        # ~8us in) are deferred until after the first xl block (see below).
        w2_sb = constp.tile([128, N_CH, 128], f16)
        w8_sb = constp.tile([128, N_CH, E], f8)
        w2_r = w2_d.rearrange("p (c e) -> p c e", e=128)
        wdef = globals().get('_W2_DEFER', 0)
        late_w = []
        for d0 in range(0, N_CH, 4):
            if wdef and d0 >= wdef:
                late_w.append((w2_sb[:, d0:d0 + 4, :], w2_r[:, d0:d0 + 4, :]))
            else:
                nc.scalar.dma_start(w2_sb[:, d0:d0 + 4, :], w2_r[:, d0:d0 + 4, :])
        if wdef:
            late_w.append((w8_sb[:], w8_d))
        else:
            nc.scalar.dma_start(w8_sb[:], w8_d)
        bias_sb = constp.tile([128, 1, E], f32)
        # combine constants: jA = [I64; I64], jB = 2^-27 * I64. Used as the
        # moving operand of per-tile "transpose" matmuls that fuse the
        # back-transpose with the hi+lo add and the scaled fp8-term add.
        jab_sb = constp.tile([128, 1, E], f32)
        setup_rest = late_w + [(bias_sb, bias_d), (jab_sb, jab_d)]
        if skip_dma:
            while setup_rest:
                sb, dr = setup_rest.pop(0)
                nc.scalar.dma_start(sb[:], dr)

        # persistent per-core tensors
        s_all = pers.tile([128, N_TILES, E], f32)      # sigmoid scores
        b_all = pers.tile([128, N_TILES, E], f32)      # biased scores
        vb_all = pers.tile([128, N_TILES, 8], f32)     # top8 of biased
        vs_all = pers.tile([128, N_TILES, 8], f32)     # top8 of masked s
        ib_all = pers.tile([128, N_TILES, 8], u32)     # indices (biased order)
        is_all = pers.tile([128, N_TILES, 8], u32)     # indices (s order)
        ibf = pers.tile([128, N_TILES, 8], f32)
        isf = pers.tile([128, N_TILES, 8], f32)
        out_s_sb = pers.tile([128, N_TILES, 8], f32)

        DMA_CH = globals().get('_DMA_CH_OVERRIDE', 4)   # h-chunks per xh piece
        DMA_CHL = globals().get('_DMA_CHL_OVERRIDE', 16)  # h-chunks per xl piece

        def supertile(pos, tok_st, mid_cb=None):
            tiles_ss = tok_st // 128
            t0 = pos // 128
            s4 = slice(t0, t0 + tiles_ss)
            xh_sb = xpool.tile([128, N_CH, tok_st], f16, tag="xh")
            xl_sb = xpool.tile([128, N_CH, tok_st], f8, tag="xl")
            foff = N_CH * pos
            xh_st = xh_d[:, foff:foff + N_CH * tok_st].rearrange(
                "p (c t) -> p c t", t=tok_st)
            xl_st = xl_d[:, foff:foff + N_CH * tok_st].rearrange(
                "p (c t) -> p c t", t=tok_st)
            if not skip_dma:
                bal = globals().get('_BAL_QUEUES', 0)
                xl_eng = nc.scalar if globals().get('_XL_ON_ACT', 1) else nc.sync
                if bal == 3:
                    # 3-ring split: xh pieces round-robin sync/ACT/pool-SWDGE,
                    # xl pieces round-robin the same three
                    engs = [nc.sync, nc.scalar, nc.gpsimd]
                    d0 = 0
                    pi = 0
                    while d0 < N_CH:
                        dn = min(DMA_CH, N_CH - d0)
                        engs[pi % 3].dma_start(
                            xh_sb[:, d0:d0 + dn, :], xh_st[:, d0:d0 + dn, :])
                        d0 += dn
                        pi += 1
                    d0 = 0
                    while d0 < N_CH:
                        dn = min(DMA_CHL // 2, N_CH - d0)
                        engs[pi % 3].dma_start(
                            xl_sb[:, d0:d0 + dn, :], xl_st[:, d0:d0 + dn, :])
                        d0 += dn
                        pi += 1
                else:
                    d0 = 0
                    pi = 0
                    while d0 < N_CH:
                        dn = min(DMA_CH, N_CH - d0)
                        eng = nc.sync if (not bal or pi % 2 == 0) else nc.scalar
                        eng.dma_start(xh_sb[:, d0:d0 + dn, :], xh_st[:, d0:d0 + dn, :])
                        d0 += dn
                        pi += 1
                    d0 = 0
                    pi = 0
                    while d0 < N_CH:
                        dn = min(DMA_CHL, N_CH - d0)
                        eng = xl_eng if (not bal or pi % 2 == 0) else nc.sync
                        eng.dma_start(xl_sb[:, d0:d0 + dn, :], xl_st[:, d0:d0 + dn, :])
                        d0 += dn
                        pi += 1
                while setup_rest:
                    sb, dr = setup_rest.pop(0)
                    nc.scalar.dma_start(sb[:], dr)
            else:
                # timing-ablation mode: mark x tiles written so the tile
                # framework doesn't assert on read-without-write
                nc.gpsimd.memset(xh_sb[:, 0, :1], 0)
                nc.gpsimd.memset(xl_sb[:, 0, :1], 0)
            if skip_compute:
                return

            # GEMM: psumA <- [wh_c|wl_c] fp16 x xh_c (rows 0:64 wh, 64:128 wl)
            #       psumB <- w8_c fp8 x xl8_c (rows 0:64)
            # both streams accumulate into one psum: fp16 terms into rows
            # 0:128 ([wh|wl] stationary), fp8 residual terms into rows 0:64
            # (same scale 2^27 by construction)
            lt_ps = ps_lt.tile([128, tok_st], f32, tag="lt")
            use_dr = globals().get('_DR', 1) and \
                tok_st >= globals().get('_DR_MIN_FD', 0)
            b_sep = globals().get('_BSEP', 1)
            mid_pos = globals().get('_MID_POS', 8)
            for c in range(N_CH):
                last_a = c == N_CH - 1 and n_terms < 3
                nc.tensor.matmul(lt_ps[:], w2_sb[:, c, :], xh_sb[:, c, :],
                                 start=(c == 0), stop=last_a)
                if c == mid_pos - 1 and mid_cb is not None:
                    # emit the previous block's J-matmuls + topk here: late
                    # enough that its PSUM->SBUF copy is done (no PE stall),
                    # early enough that its topk isn't gated on this block's
                    # remaining DMA pieces
                    mid_cb()
                if n_terms >= 3 and not use_dr and not b_sep:
                    nc.tensor.matmul(lt_ps[0:64, :], w8_sb[:, c, :],
                                     xl_sb[:, c, :],
                                     start=False, stop=(c == N_CH - 1))
            if n_terms >= 3 and not use_dr and b_sep:
                for c in range(N_CH):
                    nc.tensor.matmul(lt_ps[0:64, :], w8_sb[:, c, :],
                                     xl_sb[:, c, :],
                                     start=False, stop=(c == N_CH - 1))
            if n_terms >= 3 and use_dr:
                # fp8 DoubleRow: 2 contraction chunks per matmul
                # (out = sum_j lhsT[:, j, :].T @ rhs[:, j, :])
                for cc in range(N_CH // 2):
                    nc.tensor.matmul(
                        lt_ps[0:64, :], w8_sb[:, 2 * cc:2 * cc + 2, :],
                        xl_sb[:, 2 * cc:2 * cc + 2, :],
                        start=False, stop=(cc == N_CH // 2 - 1),
                        perf_mode=mybir.MatmulPerfMode.DoubleRow)
            if n_terms < 3:
                dummy = scr.tile([128, 1], f8, tag="dummy")
                nc.vector.tensor_copy(dummy[:], xl_sb[:, 0, :1])

            if globals().get('_DEFER_COPY', 0):
                return lt_ps
            return do_copy(tok_st, lt_ps)

        def do_copy(tok_st, lt_ps):
            lt_sb = scr.tile([128, tok_st], f32, tag="ltsb")
            cp = globals().get('_COPY_ENG', 'scalar')
            if cp == 'pool':
                nc.gpsimd.tensor_copy(lt_sb[:], lt_ps[:])
            elif cp == 'vector':
                nc.vector.tensor_copy(lt_sb[:], lt_ps[:])
            else:
                nc.scalar.copy(lt_sb[:], lt_ps[:])
            return lt_sb

        def finish_block(pos, tok_st, lt_sb, last=None):
            tiles_ss = tok_st // 128
            t0 = pos // 128
            s4 = slice(t0, t0 + tiles_ss)
            # fused back-transpose + combine: per 128-token block,
            #   l_ps[t, e] = sum_r lt[r, t] * jA[r, e] = 2^-27 * (hi + lo rows)
            # (the data block is the stationary, jA the 64-col moving)
            l_ps = ps_l.tile([128, tiles_ss, E], f32, tag="lps")
            for q in range(tiles_ss):
                qs = slice(q * 128, (q + 1) * 128)
                nc.tensor.matmul(l_ps[:, q, :], lt_sb[:, qs], jab_sb[:, 0, :],
                                 start=True, stop=True)

            s_sl = s_all[:, s4, :]
            nc.scalar.activation(s_sl, l_ps[:, :, :], Act.Sigmoid)
            if skip_topk:
                nc.vector.tensor_copy(out_s_sb[:, s4, :], s_sl[:, :, :8])
                nc.vector.tensor_copy(ib_all[:, s4, :], s_sl[:, :, 8:16])
                return
            b_sl = b_all[:, s4, :]
            beng = nc.gpsimd if globals().get('_BIAS_ON_POOL', 0) else nc.vector
            beng.tensor_tensor(
                out=b_sl, in0=s_sl,
                in1=bias_sb[:].broadcast_to([128, tiles_ss, E]),
                op=Alu.add,
            )

            for q in range(tiles_ss):
                i = t0 + q
                nc.vector.max(out=vb_all[:, i, :], in_=b_all[:, i, :])
                nc.vector.max_index(out=ib_all[:, i, :], in_max=vb_all[:, i, :],
                                    in_values=b_all[:, i, :])
            if last is not None:
                # the indices tail store needs only max_index output; issuing
                # it here lets its ~2us HBM write receipt overlap the rest of
                # the score chain (mask/2nd pass/normalize) of the last block
                od_i = out_i_d.rearrange("p (i k) -> p i k", k=8)
                nc.sync.dma_start(od_i[:, last:, :], ib_all[:, last:, :])

            # selected-expert masking: sarr = (b >= thr8) * s
            sarr = scr.tile([128, tiles_ss, E], f32, tag="sarr")
            if globals().get('_BATCH_MASK', 0):
                ge = scr.tile([128, tiles_ss, E], f32, tag="ge")
                nc.vector.tensor_tensor(
                    out=ge[:], in0=b_all[:, s4, :],
                    in1=vb_all[:, s4, 7:8].broadcast_to([128, tiles_ss, E]),
                    op=Alu.is_ge)
                nc.vector.tensor_tensor(
                    out=sarr[:], in0=ge[:], in1=s_all[:, s4, :], op=Alu.mult)
            else:
                for q in range(tiles_ss):
                    i = t0 + q
                    nc.vector.scalar_tensor_tensor(
                        out=sarr[:, q, :], in0=b_all[:, i, :],
                        scalar=vb_all[:, i, 7:8], in1=s_all[:, i, :],
                        op0=Alu.is_ge, op1=Alu.mult)

            for q in range(tiles_ss):
                i = t0 + q
                nc.vector.max(out=vs_all[:, i, :], in_=sarr[:, q, :])
                nc.vector.max_index(out=is_all[:, i, :], in_max=vs_all[:, i, :],
                                    in_values=sarr[:, q, :])

            # reorder vs_all (s-descending) into biased-rank order by idx match
            eeng = nc.gpsimd if globals().get('_EQ_ON_POOL', 0) else nc.vector
            eq = scr.tile([128, tiles_ss, 8, 8], f32, tag="eq")
            if globals().get('_EQ_U32', 1):
                eeng.tensor_tensor(
                    out=eq[:],
                    in0=ib_all[:, s4, :].broadcast_to([128, tiles_ss, 8, 8]),
                    in1=is_all[:, s4, :][:, :, None, :].broadcast_to(
                        [128, tiles_ss, 8, 8]),
                    op=Alu.is_equal,
                )
            else:
                nc.vector.tensor_copy(ibf[:, s4, :], ib_all[:, s4, :])
                nc.vector.tensor_copy(isf[:, s4, :], is_all[:, s4, :])
                eeng.tensor_tensor(
                    out=eq[:],
                    in0=ibf[:, s4, :].broadcast_to([128, tiles_ss, 8, 8]),
                    in1=isf[:, s4, :][:, :, None, :].broadcast_to(
                        [128, tiles_ss, 8, 8]),
                    op=Alu.is_equal,
                )
            g_sc = scr.tile([128, tiles_ss, 8, 8], f32, tag="g")
            eeng.tensor_tensor(
                out=g_sc[:], in0=eq[:],
                in1=vs_all[:, s4, :][:, :, None, :].broadcast_to(
                    [128, tiles_ss, 8, 8]),
                op=Alu.mult,
            )
            tsr = scr.tile([128, tiles_ss, 8], f32, tag="tsr")
            nc.vector.reduce_sum(out=tsr[:], in_=g_sc[:], axis=mybir.AxisListType.X)

            den = scr.tile([128, tiles_ss], f32, tag="den")
            nc.vector.reduce_sum(out=den[:], in_=vs_all[:, s4, :],
                                 axis=mybir.AxisListType.X)
            rec = scr.tile([128, tiles_ss], f32, tag="rec")
            nc.vector.reciprocal(rec[:], den[:])
            nc.vector.scalar_tensor_tensor(
                out=out_s_sb[:, s4, :], in0=tsr[:], scalar=ROUTE_SCALE,
                in1=rec[:].broadcast_to([128, tiles_ss, 8]),
                op0=Alu.mult, op1=Alu.mult,
            )
            if globals().get('_OUT_PER_ST', 0):
                od_s = out_s_d.rearrange("p (i k) -> p i k", k=8)
                od_i = out_i_d.rearrange("p (i k) -> p i k", k=8)
                nc.scalar.dma_start(od_s[:, s4, :], out_s_sb[:, s4, :])
                nc.scalar.dma_start(od_i[:, s4, :], ib_all[:, s4, :])

        schedule = globals().get('_SCHED', SCHEDULE)
        assert sum(schedule) == T_CORE

        def whole_pass():
            pos = 0
            pos_fin = 0
            tail0 = 0
            n_early = globals().get('_EARLY_TILES', 15)
            hi_last = globals().get('_HI_LAST', 0)
            defer = globals().get('_DEFER_J', 1)
            early_done = False
            pending = None          # (pos, tok_st, lt_sb) not yet finished
            store_out = not skip_compute and not skip_topk and \
                not globals().get('_OUT_PER_ST', 0)

            def maybe_early_store():
                nonlocal early_done, tail0
                if store_out and not early_done and pos_fin >= 128 * n_early:
                    # store the finished head tiles while the x stream still
                    # runs; only the short tail rides the final store pair
                    e = pos_fin // 128
                    od_s = out_s_d.rearrange("p (i k) -> p i k", k=8)
                    od_i = out_i_d.rearrange("p (i k) -> p i k", k=8)
                    nc.scalar.dma_start(od_s[:, 0:e, :], out_s_sb[:, 0:e, :])
                    nc.scalar.dma_start(od_i[:, 0:e, :], ib_all[:, 0:e, :])
                    early_done = True
                    tail0 = e

            warm = globals().get('_PE_WARM', 0)
            if warm and not skip_compute:
                # HAM keep-warm filler: PE idles ~5us at each iteration start
                # (all-engine loop barrier + first block's DMA), long enough
                # for the clock gate to re-throttle to 1.2 GHz. Issue dummy
                # matmuls with no DMA dependency to span the gap and hold the
                # 2.4 GHz clock. Sized to finish before the first block lands.
                wm_ps = ps_lt.tile([128, 128], f32, tag="lt")
                for _ in range(warm):
                    nc.tensor.matmul(wm_ps[:], w2_sb[:, 0, :], w2_sb[:, 0, :],
                                     start=True, stop=True)

            midj = globals().get('_MID_J', 0)
            for bi, tok_st in enumerate(schedule):
                hp = hi_last and bi >= len(schedule) - hi_last

                def mid_cb():
                    # fires between fp16 chunk mid_pos-1 and mid_pos of the
                    # current block: the previous block's J+topk lands mid-GEMM
                    nonlocal pending, pos_fin
                    if pending is not None:
                        p_pos, p_tok, p_lt = pending
                        if globals().get('_DEFER_COPY', 0):
                            p_lt = do_copy(p_tok, p_lt)
                        finish_block(p_pos, p_tok, p_lt)
                        pos_fin = p_pos + p_tok
                        pending = None

                use_mid = bool(defer and midj and not skip_compute)
                with tc.high_priority() if hp else _nullctx():
                    lt_sb = supertile(pos, tok_st,
                                      mid_cb=mid_cb if use_mid else None)
                if not skip_compute:
                    if defer:
                        if pending is not None:
                            p_pos, p_tok, p_lt = pending
                            if globals().get('_DEFER_COPY', 0):
                                p_lt = do_copy(p_tok, p_lt)
                            finish_block(p_pos, p_tok, p_lt)
                            pos_fin = p_pos + p_tok
                        pending = (pos, tok_st, lt_sb)
                    else:
                        finish_block(pos, tok_st, lt_sb)
                        pos_fin = pos + tok_st
                pos += tok_st
                maybe_early_store()
            idx_tail_done = None
            if pending is not None:
                p_pos, p_tok, p_lt = pending
                # if the stored tail lies entirely within the last block,
                # fire its indices store early (right after max_index)
                if store_out and globals().get('_EARLY_IDX_TAIL', 1) \
                        and n_early >= p_pos // 128:
                    idx_tail_done = max(p_pos // 128, n_early)
                with tc.high_priority() if globals().get('_HI_FLUSH', 0) \
                        else _nullctx():
                    if globals().get('_DEFER_COPY', 0):
                        p_lt = do_copy(p_tok, p_lt)
                    finish_block(p_pos, p_tok, p_lt, last=idx_tail_done)
                pos_fin = p_pos + p_tok
                maybe_early_store()
            if not skip_compute and not skip_topk and not globals().get('_OUT_PER_ST', 0):
                t0 = tail0 if early_done else 0
                od_s = out_s_d.rearrange("p (i k) -> p i k", k=8)
                od_i = out_i_d.rearrange("p (i k) -> p i k", k=8)
                ieng = nc.sync if globals().get('_STORE_SPLIT', 1) else nc.scalar
                nc.scalar.dma_start(od_s[:, t0:, :], out_s_sb[:, t0:, :])
                # indices tail may already be (partially) stored by the last
                # finish_block's early-idx store, which covered [ie, N_TILES)
                ie = idx_tail_done if idx_tail_done is not None else N_TILES
                if ie > t0:
                    ieng.dma_start(od_i[:, t0:ie, :], ib_all[:, t0:ie, :])

        if reps == 1:
            whole_pass()
        else:
            with tc.For_i(0, reps, 1,
                          staggered_reset=bool(globals().get('_STAGGER', 0))):
                whole_pass()


def build_nc(reps=1, skip_dma=False, skip_compute=False, n_terms=3, skip_topk=False):
    nc = bacc.Bacc("TRN2", target_bir_lowering=False, debug=False)
    xh_d = nc.dram_tensor("xh_d", [128, N_CH * T_CORE], f16, kind="ExternalInput")
    xl_d = nc.dram_tensor("xl_d", [128, N_CH * T_CORE], f8, kind="ExternalInput")
    w2_d = nc.dram_tensor("w2_d", [128, N_CH * 128], f16, kind="ExternalInput")
    w8_d = nc.dram_tensor("w8_d", [128, N_CH * E], f8, kind="ExternalInput")
    bias_d = nc.dram_tensor("bias_d", [128, E], f32, kind="ExternalInput")
    jab_d = nc.dram_tensor("jab_d", [128, E], f32, kind="ExternalInput")
    out_s_d = nc.dram_tensor("out_s_d", [128, N_TILES * 8], f32, kind="ExternalOutput")
    out_i_d = nc.dram_tensor("out_i_d", [128, N_TILES * 8], u32, kind="ExternalOutput")

    with tile.TileContext(nc) as tc:
        router_body(
            tc,
            (out_s_d.ap(), out_i_d.ap()),
            (xh_d.ap(), xl_d.ap(), w2_d.ap(), w8_d.ap(), bias_d.ap(), jab_d.ap()),
            reps=reps, skip_dma=skip_dma, skip_compute=skip_compute,
            n_terms=n_terms, skip_topk=skip_topk,
        )
    nc.compile()
    return nc


def pack_x_shard(xT, dtype):
    """[H, T_CORE] -> [128, N_CH*T_CORE] with each SCHEDULE block stored
    contiguously: out[p, N_CH*pos + c*tok_st + t] = xT[c*128 + p, pos + t]."""
    v = xT.reshape(N_CH, 128, T_CORE)
    blocks = []
    pos = 0
    for tok_st in globals().get('_SCHED', SCHEDULE):
        blk = v[:, :, pos:pos + tok_st]            # [N_CH, 128, tok_st]
        blocks.append(blk.transpose(1, 0, 2).reshape(128, N_CH * tok_st))
        pos += tok_st
    return np.ascontiguousarray(np.concatenate(blocks, axis=1)).astype(dtype)


def pack_w2(wh, wl):
    """wh/wl [E, H] fp16 -> [128, N_CH*128] with wh in cols 0:64, wl in 64:128
    of each chunk: out[p, c*128 + e] = (wh if e < E else wl)[e % E, c*128 + p]."""
    vh = wh.T.reshape(N_CH, 128, E)
    vl = wl.T.reshape(N_CH, 128, E)
    v = np.concatenate([vh, vl], axis=2)          # [N_CH, 128, 128]
    return np.ascontiguousarray(v.transpose(1, 0, 2)).reshape(128, N_CH * 128)


def pack_w8(w):
    """w [E, H] f32 -> e4m3 [128, N_CH*E]: out[p, c*E + e] = w8[e, c*128+p]."""
    w8 = (w * 2.0 ** WB).astype(ml_dtypes.float8_e4m3)
    v = w8.T.reshape(N_CH, 128, E)
    return np.ascontiguousarray(v.transpose(1, 0, 2)).reshape(128, N_CH * E)


_NC_CACHE = {}


def host_pack(hidden_states, expert_bias, gate_w):
    x2 = np.asarray(hidden_states, dtype=np.float32).reshape(T_FULL, H)
    w = np.asarray(gate_w, dtype=np.float32)
    bias = np.asarray(expert_bias, dtype=np.float32)

    xh0 = x2.astype(np.float16)
    r = (x2 - xh0.astype(np.float32)) * float(2.0 ** XA)
    xh = (xh0.astype(np.float32) * float(2.0 ** XS)).astype(np.float16)
    ws = float(2.0 ** WS)
    wh = (w.astype(np.float16).astype(np.float32) * ws).astype(np.float16)
    wl = ((w - w.astype(np.float16).astype(np.float32)) * ws).astype(np.float16)

    w2_p = pack_w2(wh, wl)
    w8_p = pack_w8(w)
    bias_p = np.ascontiguousarray(np.broadcast_to(bias[None, :], (128, E)))
    jab = np.zeros((128, E), dtype=np.float32)
    jab[0:E, :] = np.eye(E) * CSCALE
    jab[E:2 * E, :] = np.eye(E) * CSCALE
    jab_p = jab

    in_maps = []
    for k in range(N_CORES):
        rows = slice(k * T_CORE, (k + 1) * T_CORE)
        in_maps.append({
            "xh_d": pack_x_shard(np.ascontiguousarray(xh[rows].T), np.float16),
            "xl_d": pack_x_shard(np.ascontiguousarray(r[rows].T.astype(np.float32)),
                                 ml_dtypes.float8_e4m3),
            "w2_d": w2_p,
            "w8_d": w8_p,
            "bias_d": bias_p,
            "jab_d": jab_p,
        })
    return in_maps


def kernel(hidden_states, expert_bias, gate_w):
    in_maps = host_pack(hidden_states, expert_bias, gate_w)

    if "nc" not in _NC_CACHE:
        _NC_CACHE["nc"] = build_nc()
    nc = _NC_CACHE["nc"]

    res = bass_utils.run_bass_kernel_spmd(nc, in_maps, core_ids=list(range(N_CORES)))

    scores = np.empty((T_FULL, TOPK), dtype=np.float32)
    idx = np.empty((T_FULL, TOPK), dtype=np.int32)
    for k in range(N_CORES):
        o = res.results[k]
        s = o["out_s_d"].reshape(128, N_TILES, TOPK).transpose(1, 0, 2).reshape(T_CORE, TOPK)
        i = o["out_i_d"].view(np.int32).reshape(128, N_TILES, TOPK).transpose(1, 0, 2).reshape(T_CORE, TOPK)
        scores[k * T_CORE:(k + 1) * T_CORE] = s
        idx[k * T_CORE:(k + 1) * T_CORE] = i
    return scores, idx

